# revision 45
# baseline (speedup 1.0000x reference)
"""Trainium2 Bass kernel for nn_DiagonalSSM (4-layer diagonal-SSM LM).

Sharding (8 cores):
  - Trunk: token-sharded. Core k handles batch k//4, tokens
    [(k%4)*512, (k%4+1)*512). The SSM scan runs as chunked scaled-cumsums on
    the PE; cross-segment carries use one tiny AllGather (2 groups of 4) per
    layer, launched early (G computed directly from Bu) so it overlaps the
    scan itself.
  - Head: vocab-sharded. After the final LN, activations are AllGathered
    (all 8 cores, split into two halves for overlap) and each core computes
    logits for its 4000-vocab slice over all 4096 tokens.

Layout: activations feature-major ([d, t]); residual stream kept in f32r.
All large matmuls run in float32r (full PE rate, ~13-bit mantissa); the
scan cumsum runs in fp32.
"""

import ml_dtypes
import numpy as np

L, D, S, V = 4, 512, 256, 32000
DFF = 1368
B, T = 2, 2048
NCORES = 8
NSEG = 4
TSEG = 512
VSH = V // NCORES  # 4000
NVC = 8
VC = VSH // NVC    # 500
EPS = 1e-5
CH = 128
NCH = TSEG // CH   # 4
ND = D // 128      # 4
NS = S // 128      # 2
NFT = (DFF + 127) // 128  # 11
FTS = [128] * (DFF // 128) + ([DFF % 128] if DFF % 128 else [])
NLC = 6 * L + 2    # packed LN-param columns

_NC_CACHE = {}


def _build_nc(debug=False):
    import concourse.bass as bass
    import concourse.tile as tile
    from concourse import bacc, mybir
    from concourse.masks import make_identity

    f32 = mybir.dt.float32
    f32r = mybir.dt.float32r
    i32 = mybir.dt.int32
    AF = mybir.ActivationFunctionType
    OP = mybir.AluOpType

    nc = bacc.Bacc("TRN2", target_bir_lowering=False, debug=False,
                   num_devices=NCORES)

    # ---------------- DRAM I/O ----------------
    d_x = nc.dram_tensor("x_seg", [TSEG, 1], i32, kind="ExternalInput")
    d_emb = nc.dram_tensor("emb", [V, D], f32, kind="ExternalInput")
    d_pos = nc.dram_tensor("pos_seg", [TSEG, D], f32, kind="ExternalInput")
    d_BwT = nc.dram_tensor("BwT", [L, D, S], f32r, kind="ExternalInput")
    d_CwT = nc.dram_tensor("CwT", [L, S, D], f32r, kind="ExternalInput")
    d_w1T = nc.dram_tensor("w1T", [L, D, DFF], f32r, kind="ExternalInput")
    d_w2T = nc.dram_tensor("w2T", [L, D, DFF], f32r, kind="ExternalInput")
    d_w3T = nc.dram_tensor("w3T", [L, DFF, D], f32r, kind="ExternalInput")
    bf16 = mybir.dt.bfloat16
    d_hWTb = nc.dram_tensor("headWTb", [D, VSH], bf16, kind="ExternalInput")
    d_hbb = nc.dram_tensor("headb_bc", [128, VSH], f32, kind="ExternalInput")
    d_lnc = nc.dram_tensor("lncols", [D, NLC], f32, kind="ExternalInput")
    d_laminv = nc.dram_tensor("laminv", [L, CH, S], f32, kind="ExternalInput")
    d_lamp = nc.dram_tensor("lamp", [L, CH, S], f32, kind="ExternalInput")
    d_chc = nc.dram_tensor("chc", [L, CH, S], f32, kind="ExternalInput")
    d_lamwB = nc.dram_tensor("lamwB", [L, CH, NCH, S], f32,
                             kind="ExternalInput")
    d_chc2T = nc.dram_tensor("chc2T", [L, S, TSEG], f32, kind="ExternalInput")
    d_lamc = nc.dram_tensor("lamc", [L, NCH, NCH, S], f32, kind="ExternalInput")
    d_segcT = nc.dram_tensor("segcoefT", [L, 128, NS * NCH], f32,
                             kind="ExternalInput")
    d_U = nc.dram_tensor("Utri", [CH, CH], f32, kind="ExternalInput")
    d_ones4 = nc.dram_tensor("ones4", [NCH, 128], f32, kind="ExternalInput")
    d_ones4f = nc.dram_tensor("ones4f", [NCH, TSEG], f32, kind="ExternalInput")
    d_ones128r = nc.dram_tensor("ones128r", [128, 128], f32r, kind="ExternalInput")
    d_onesD = nc.dram_tensor("onesD", [128, 128], f32r, kind="ExternalInput")

    d_out = nc.dram_tensor("logits", [B * T, VSH], bf16, kind="ExternalOutput")

    dbg = {}
    if debug:
        for nm, shp in (
            ("dbg_h0", [D, TSEG]), ("dbg_xn", [D, TSEG]),
            ("dbg_bu", [CH, NCH * S]), ("dbg_loc", [CH, NCH * S]),
            ("dbg_hst", [S, TSEG]), ("dbg_h1", [D, TSEG]),
            ("dbg_xnf", [D, TSEG]),
        ):
            dbg[nm] = nc.dram_tensor(nm, shp, f32, kind="ExternalOutput")

    with tile.TileContext(nc) as tc:
        with (
            tc.tile_pool(name="const", bufs=1) as cpool,
            tc.tile_pool(name="hm", bufs=1) as hm,
            tc.tile_pool(name="scr", bufs=2) as scr,
            tc.tile_pool(name="one", bufs=1) as one,
            tc.tile_pool(name="ps", bufs=1, space="PSUM") as ps,
            tc.tile_pool(name="ps4", bufs=4, space="PSUM") as ps4,
            tc.tile_pool(name="dram", bufs=1, space="DRAM") as dram,
        ):
            # ---------------- constants ----------------
            ident = cpool.tile([128, 128], f32, name="ident")
            make_identity(nc, ident[:, :])
            U_sb = cpool.tile([CH, CH], f32, name="U_sb")
            nc.sync.dma_start(out=U_sb[:, :], in_=d_U[:, :])
            ones4_sb = cpool.tile([NCH, 128], f32, name="ones4_sb")
            nc.sync.dma_start(out=ones4_sb[:, :], in_=d_ones4[:, :])
            ones4f_sb = cpool.tile([NCH, TSEG], f32, name="ones4f_sb")
            nc.sync.dma_start(out=ones4f_sb[:, :], in_=d_ones4f[:, :])
            ones128r_sb = cpool.tile([128, 128], f32r, name="ones128r_sb")
            nc.sync.dma_start(out=ones128r_sb[:, :], in_=d_ones128r[:, :])
            onesD_sb = cpool.tile([128, 128], f32r, name="onesD_sb")
            nc.sync.dma_start(out=onesD_sb[:, :], in_=d_onesD[:, :])
            eps_sb = cpool.tile([128, 1], f32, name="eps_sb")
            nc.vector.memset(eps_sb[:, :], EPS)
            lnc_sb = []
            for dd in range(ND):
                t = cpool.tile([128, NLC], f32, tag=f"lnc{dd}", name=f"lnc{dd}")
                nc.sync.dma_start(
                    out=t[:, :], in_=d_lnc[dd * 128:(dd + 1) * 128, :])
                lnc_sb.append(t)

            def lncol(key, ll, dd):
                base = {"n1w": 0, "n1b": L, "n2w": 2 * L, "n2b": 3 * L,
                        "Dpw": 4 * L, "Dpb": 5 * L + 2}
                if key == "noww":
                    c = 5 * L
                elif key == "nob":
                    c = 5 * L + 1
                else:
                    c = base[key] + ll
                return lnc_sb[dd][:, c:c + 1]

            # ---------------- h master (feature-major, f32r) --------------
            h = [hm.tile([128, TSEG], f32r, tag=f"h{dd}", name=f"h{dd}")
                 for dd in range(ND)]
            act_ctx = tc.tile_pool(name="act", bufs=1)
            act = act_ctx.__enter__()

            # ---------------- embedding ----------------
            with tc.tile_pool(name="emb", bufs=1) as embp:
                idx_t, e_tl, p_tl = [], [], []
                for tt in range(NCH):
                    idx_sb = embp.tile([128, 1], i32, tag=f"idx{tt}",
                                       name=f"idx{tt}")
                    nc.sync.dma_start(
                        out=idx_sb[:, :], in_=d_x[tt * 128:(tt + 1) * 128, :])
                    idx_t.append(idx_sb)
                for tt in range(NCH):
                    e_t = embp.tile([128, D], f32, tag=f"e_t{tt}",
                                    name=f"e_t{tt}")
                    nc.gpsimd.indirect_dma_start(
                        out=e_t[:, :], out_offset=None, in_=d_emb[:, :],
                        in_offset=bass.IndirectOffsetOnAxis(
                            ap=idx_t[tt][:, :1], axis=0),
                    )
                    e_tl.append(e_t)
                    p_t = embp.tile([128, D], f32, tag=f"p_t{tt}",
                                    name=f"p_t{tt}")
                    nc.sync.dma_start(
                        out=p_t[:, :], in_=d_pos[tt * 128:(tt + 1) * 128, :])
                    p_tl.append(p_t)
                for tt in range(NCH):
                    htm = embp.tile([128, D], f32, tag=f"htm{tt}",
                                    name=f"htm{tt}")
                    eng = nc.vector if tt % 2 == 0 else nc.gpsimd
                    eng.tensor_tensor(
                        out=htm[:, :], in0=e_tl[tt][:, :], in1=p_tl[tt][:, :],
                        op=OP.add)
                    for dd in range(ND):
                        trp = ps.tile([128, 128], f32,
                                      tag=("sm" if dd % 2 == 0 else "gps"),
                                      name="trp")
                        nc.tensor.transpose(
                            trp[:, :], htm[:, dd * 128:(dd + 1) * 128],
                            ident[:, :])
                        if (tt + dd) % 2 == 0:
                            nc.vector.tensor_copy(
                                out=h[dd][:, tt * 128:(tt + 1) * 128],
                                in_=trp[:, :])
                        else:
                            nc.scalar.activation(
                                out=h[dd][:, tt * 128:(tt + 1) * 128],
                                in_=trp[:, :], func=AF.Copy)

            def dump_fm(key, tiles):
                for dd in range(len(tiles)):
                    o = one.tile([128, TSEG], f32, tag="dbgcp", name="dbgcp")
                    nc.vector.tensor_copy(out=o[:, :], in_=tiles[dd][:, :])
                    nc.sync.dma_start(
                        out=dbg[key][dd * 128:(dd + 1) * 128, :], in_=o[:, :])

            if debug:
                dump_fm("dbg_h0", h)

            # ---------------- LN helper ----------------
            def ln_stats():
                # mean and E[x^2] accumulate in parallel (separate banks)
                mu = ps.tile([128, TSEG], f32, tag="sm", name="mu")
                ex2 = ps.tile([128, TSEG], f32, tag="gps", name="ex2")
                for dd in range(ND):
                    nc.tensor.matmul(mu[:, :], onesD_sb[:, :], h[dd][:, :],
                                     start=(dd == 0), stop=(dd == ND - 1))
                for dd in range(ND):
                    h2 = scr.tile([128, TSEG], f32r, tag="h2", name="h2")
                    eng = nc.vector if dd % 2 == 0 else nc.gpsimd
                    eng.tensor_tensor(
                        out=h2[:, :], in0=h[dd][:, :], in1=h[dd][:, :], op=OP.mult)
                    nc.tensor.matmul(ex2[:, :], onesD_sb[:, :], h2[:, :],
                                     start=(dd == 0), stop=(dd == ND - 1))
                # rstd chain reads mu straight from PSUM; mu_sb copy runs
                # concurrently on Pool
                mu_sb = one.tile([128, TSEG], f32, tag="mu_sb", name="mu_sb")
                nc.scalar.activation(out=mu_sb[:, :], in_=mu[:, :],
                                     func=AF.Copy)
                var = one.tile([128, TSEG], f32, tag="var_sb", name="var_sb")
                nc.vector.tensor_tensor(
                    out=var[:, :], in0=mu[:, :], in1=mu_sb[:, :], op=OP.mult)
                nc.vector.tensor_tensor(
                    out=var[:, :], in0=ex2[:, :], in1=var[:, :], op=OP.subtract)
                sd = one.tile([128, TSEG], f32, tag="sd", name="sd")
                nc.scalar.activation(out=sd[:, :], in_=var[:, :], func=AF.Sqrt,
                                     bias=eps_sb[:, :], scale=1.0)
                rstd = one.tile([128, TSEG], f32, tag="rstd", name="rstd")
                nc.vector.reciprocal(out=rstd[:, :], in_=sd[:, :])
                return mu_sb, sd, rstd

            def layer_norm(w_key, b_key, ll=None, out_tag="xn", out_dtype=None):
                mu_sb, sd, rstd = ln_stats()
                hc = []
                for dd in range(ND):
                    c = act.tile([128, TSEG], f32r, tag=f"hc{dd}", name=f"hc{dd}")
                    eng = nc.vector if dd < 2 else nc.gpsimd
                    eng.tensor_tensor(
                        out=c[:, :], in0=h[dd][:, :], in1=mu_sb[:, :],
                        op=OP.subtract)
                    hc.append(c)
                xn = []
                for dd in range(ND):
                    eng = nc.vector if dd < 2 else nc.gpsimd
                    t1 = scr.tile([128, TSEG], f32, tag="lnt1", name="lnt1")
                    eng.tensor_tensor(
                        out=t1[:, :], in0=hc[dd][:, :], in1=rstd[:, :], op=OP.mult)
                    xo = act.tile([128, TSEG], out_dtype or f32r,
                                  tag=f"{out_tag}{dd}", name=f"{out_tag}{dd}")
                    eng.tensor_scalar(
                        out=xo[:, :], in0=t1[:, :],
                        scalar1=lncol(w_key, ll, dd), scalar2=lncol(b_key, ll, dd),
                        op0=OP.mult, op1=OP.add)
                    xn.append(xo)
                return xn

            # ---------------- layers ----------------
            with (
                tc.tile_pool(name="tabs", bufs=1) as tabs,
                tc.tile_pool(name="wbig", bufs=1) as wbig,
            ):
                for ll in range(L):
                    laminv_sb = tabs.tile([CH, S], f32, tag="laminv", name="laminv")
                    lamp_sb = tabs.tile([CH, S], f32, tag="lamp", name="lamp")
                    chc_sb = tabs.tile([CH, S], f32, tag="chc", name="chc")
                    lamwB_sb = tabs.tile([CH, NCH, S], f32, tag="lamwB",
                                         name="lamwB")
                    chc2T_sb = [tabs.tile([128, TSEG], f32, tag=f"chc2T{ss}",
                                          name=f"chc2T{ss}") for ss in range(NS)]
                    lamc_sb = tabs.tile([NCH, NCH, S], f32, tag="lamc", name="lamc")
                    segcT_sb = tabs.tile([128, NS * NCH], f32, tag="segcT",
                                         name="segcT")
                    nc.sync.dma_start(out=laminv_sb[:, :], in_=d_laminv[ll])
                    nc.sync.dma_start(out=lamp_sb[:, :], in_=d_lamp[ll])
                    nc.sync.dma_start(out=chc_sb[:, :], in_=d_chc[ll])
                    nc.sync.dma_start(out=lamwB_sb[:, :, :], in_=d_lamwB[ll])
                    for ss in range(NS):
                        nc.sync.dma_start(
                            out=chc2T_sb[ss][:, :],
                            in_=d_chc2T[ll, ss * 128:(ss + 1) * 128, :])
                    nc.sync.dma_start(out=lamc_sb[:, :, :], in_=d_lamc[ll])
                    nc.sync.dma_start(out=segcT_sb[:, :], in_=d_segcT[ll])
                    BwT_sb = []
                    for dd in range(ND):
                        t = tabs.tile([128, S], f32r, tag=f"BwT{dd}",
                                      name=f"BwT{dd}")
                        nc.sync.dma_start(
                            out=t[:, :], in_=d_BwT[ll, dd * 128:(dd + 1) * 128, :])
                        BwT_sb.append(t)
                    CwT_sb = []
                    for ss in range(NS):
                        t = tabs.tile([128, D], f32r, tag=f"CwT{ss}",
                                      name=f"CwT{ss}")
                        nc.sync.dma_start(
                            out=t[:, :], in_=d_CwT[ll, ss * 128:(ss + 1) * 128, :])
                        CwT_sb.append(t)
                    w1sb, w2sb = [], []
                    for dd in range(ND):
                        t = wbig.tile([128, DFF], f32r, tag=f"w1sb{dd}",
                                      name=f"w1sb{dd}")
                        nc.sync.dma_start(
                            out=t[:, :], in_=d_w1T[ll, dd * 128:(dd + 1) * 128, :])
                        w1sb.append(t)
                        t = wbig.tile([128, DFF], f32r, tag=f"w2sb{dd}",
                                      name=f"w2sb{dd}")
                        nc.sync.dma_start(
                            out=t[:, :], in_=d_w2T[ll, dd * 128:(dd + 1) * 128, :])
                        w2sb.append(t)

                    # per-layer folded Bw weights (off critical path)
                    BwTw = []
                    for dd in range(ND):
                        t = tabs.tile([128, S], f32r, tag=f"BwTw{dd}",
                                      name=f"BwTw{dd}")
                        nc.gpsimd.tensor_scalar_mul(
                            out=t[:, :], in0=BwT_sb[dd][:, :],
                            scalar1=lncol("n1w", ll, dd))
                        BwTw.append(t)

                    # ---- Bb[s] = sum_d n1b[d] BwT[d,s]; broadcast + pre-scale
                    n1br = []
                    for dd in range(ND):
                        t = scr.tile([128, 1], f32r, tag="n1br", name="n1br")
                        nc.vector.tensor_copy(out=t[:, :],
                                              in_=lncol("n1b", ll, dd))
                        n1br.append(t)
                    colb = ps.tile([CH, S], f32, tag="gps", name="colb")
                    for dd in range(ND):
                        nc.tensor.matmul(colb[0:1, :], n1br[dd][:, :],
                                         BwT_sb[dd][:, :],
                                         start=(dd == 0), stop=(dd == ND - 1))
                    Bb_row = one.tile([1, S], f32, tag="Bb_row", name="Bb_row")
                    nc.vector.tensor_copy(out=Bb_row[:, :], in_=colb[0:1, :])
                    # broadcast Bb_row across partitions on the PE
                    bb_ps = ps.tile([CH, S], f32, tag="pb", name="bb_ps")
                    nc.tensor.matmul(bb_ps[:, :], ones4_sb[0:1, :],
                                     Bb_row[:, :], start=True, stop=True)
                    Bblam = one.tile([CH, S], f32, tag="Bblam", name="Bblam")
                    nc.vector.tensor_tensor(
                        out=Bblam[:, :], in0=bb_ps[:, :], in1=laminv_sb[:, :],
                        op=OP.mult)

                    # ---- LN1 stats ----
                    mu_sb, sd, rstd = ln_stats()

                    # rstd as per-token columns (PE transpose of bcast rows)
                    sd_col = []
                    for c in range(NCH):
                        trp = ps.tile([128, 128], f32,
                                      tag=("sm" if c % 2 == 0 else "pa"),
                                      name="trpr")
                        nc.tensor.transpose(
                            trp[:, :], rstd[:, c * 128:(c + 1) * 128],
                            ident[:, :])
                        col = scr.tile([128, 1], f32, tag="sdc", name="sdc",
                                       bufs=4)
                        if c % 2 == 0:
                            nc.vector.tensor_copy(out=col[:, :], in_=trp[:, 0:1])
                        else:
                            nc.scalar.activation(out=col[:, :], in_=trp[:, 0:1],
                                                 func=AF.Copy)
                        sd_col.append(col)

                    # hc, produced chunk-by-chunk so P-matmuls start early
                    hc = [act.tile([128, TSEG], f32r, tag=f"hc{dd}",
                                   name=f"hc{dd}") for dd in range(ND)]
                    for c in range(NCH):
                        for dd in range(ND):
                            eng = nc.vector if (c + dd) % 2 == 0 else nc.gpsimd
                            eng.tensor_tensor(
                                out=hc[dd][:, c * 128:(c + 1) * 128],
                                in0=h[dd][:, c * 128:(c + 1) * 128],
                                in1=mu_sb[:, c * 128:(c + 1) * 128],
                                op=OP.subtract)

                    # ---- Bu chunks: v = (hc@BwTw)*laminv*rstd + Bb*laminv ----
                    v_all = act.tile([CH, NCH, S], f32, tag="v_all", name="v_all")
                    gacc = ps.tile([CH, S], f32, tag="gps", name="gacc")
                    for c in range(NCH):
                        ec = nc.vector if c % 2 == 0 else nc.gpsimd
                        bu_ps = ps.tile([128, TSEG], f32,
                                        tag=("pa" if c % 2 == 0 else "pb"),
                                        name="bu_ps")
                        for dd in range(ND):
                            nc.tensor.matmul(
                                bu_ps[:, :S],
                                hc[dd][:, c * 128:(c + 1) * 128],
                                BwTw[dd][:, :],
                                start=(dd == 0), stop=(dd == ND - 1))
                        vt = scr.tile([CH, S], f32, tag="vt", name="vt", bufs=4)
                        nc.vector.tensor_tensor(
                            out=vt[:, :], in0=bu_ps[:, :S], in1=laminv_sb[:, :],
                            op=OP.mult)
                        ec.tensor_scalar_mul(
                            out=vt[:, :], in0=vt[:, :],
                            scalar1=sd_col[c][:, :])
                        ec.tensor_tensor(
                            out=v_all[:, c, :], in0=vt[:, :], in1=Bblam[:, :],
                            op=OP.add)
                        # G partial: lamw-scaled column-sum, PSUM-accumulated
                        vw = scr.tile([CH, S], f32, tag="vw", name="vw", bufs=4)
                        ec.tensor_tensor(
                            out=vw[:, :], in0=v_all[:, c, :],
                            in1=lamwB_sb[:, c, :], op=OP.mult)
                        nc.tensor.matmul(gacc[0:1, :], U_sb[:, 127:128],
                                         vw[:, :], start=(c == 0),
                                         stop=(c == NCH - 1))

                    # launch AllGather of local-final state ASAP
                    G_sb = one.tile([1, S], f32, tag="G_sb", name="G_sb")
                    nc.scalar.activation(out=G_sb[:, :], in_=gacc[0:1, :],
                                         func=AF.Copy)
                    g_in = dram.tile([1, S], f32, tag="g_in", name="g_in")
                    g_out = dram.tile([NCH, S], f32, tag="g_out", name="g_out")
                    nc.sync.dma_start(out=g_in[:, :], in_=G_sb[:, :])
                    nc.gpsimd.collective_compute(
                        "AllGather", mybir.AluOpType.bypass,
                        replica_groups=[[0, 1, 2, 3], [4, 5, 6, 7]],
                        ins=[g_in.opt()], outs=[g_out.opt()],
                    )

                    # ---- intra-chunk cumsums (overlap the collective) ----
                    intra = act.tile([CH, NCH, S], f32, tag="intra", name="intra")
                    for c in range(NCH):
                        cum = ps.tile([CH, S], f32, tag="sm", name="cum")
                        nc.tensor.matmul(cum[:, :], U_sb[:, :], v_all[:, c, :],
                                         start=True, stop=True)
                        nc.vector.tensor_tensor(
                            out=intra[:, c, :], in0=cum[:, :], in1=lamp_sb[:, :],
                            op=OP.mult)

                    # ---- transpose chunk 0 (carry-free) while fixups run ----
                    hsT = [act.tile([128, TSEG], f32r, tag=f"hsT{ss}",
                                    name=f"hsT{ss}") for ss in range(NS)]

                    def transpose_chunk(c):
                        for ss in range(NS):
                            trp = ps.tile([128, 128], f32,
                                          tag=("sm" if (c * NS + ss) % 2 == 0
                                               else "gps"), name="trp2")
                            nc.tensor.transpose(
                                trp[:, :], intra[:, c, ss * 128:(ss + 1) * 128],
                                ident[:, :])
                            if ss % 2 == 0:
                                nc.vector.tensor_copy(
                                    out=hsT[ss][:, c * 128:(c + 1) * 128],
                                    in_=trp[:, :])
                            else:
                                nc.scalar.activation(
                                    out=hsT[ss][:, c * 128:(c + 1) * 128],
                                    in_=trp[:, :], func=AF.Copy)

                    transpose_chunk(0)

                    # ---- chunk-carry fixup (local), batched ----
                    S4 = one.tile([NCH, S], f32, tag="S4", name="S4")
                    nc.sync.dma_start(out=S4[:, :], in_=intra[CH - 1:CH, :, :])
                    rows_c, pfix_c = {}, {}
                    for c in range(1, NCH):
                        rows = scr.tile([NCH, S], f32, tag="rows", name="rows",
                                        bufs=3)
                        eng = nc.vector if c % 2 == 0 else nc.gpsimd
                        eng.tensor_tensor(
                            out=rows[:, :], in0=S4[:, :], in1=lamc_sb[:, c, :],
                            op=OP.mult)
                        rows_c[c] = rows
                    for c in range(1, NCH):
                        pfix = ps.tile([CH, S], f32,
                                       tag=("pa" if c % 2 == 0 else "pb"),
                                       name="pfix")
                        nc.tensor.matmul(pfix[:, :], ones4_sb[:, :],
                                         rows_c[c][:, :], start=True, stop=True)
                        pfix_c[c] = pfix
                    for c in range(1, NCH):
                        tmp = scr.tile([CH, S], f32, tag="fixt", name="fixt",
                                       bufs=3)
                        nc.vector.tensor_tensor(
                            out=tmp[:, :], in0=pfix_c[c][:, :], in1=chc_sb[:, :],
                            op=OP.mult)
                        eng = nc.vector if c % 2 == 0 else nc.gpsimd
                        eng.tensor_tensor(
                            out=intra[:, c, :], in0=intra[:, c, :], in1=tmp[:, :],
                            op=OP.add)
                        transpose_chunk(c)

                    if debug and ll == 0:
                        o3 = one.tile([128, NCH * S], f32, tag="dbgcp3",
                                      name="dbgcp3")
                        nc.vector.tensor_copy(
                            out=o3[:, :],
                            in_=intra[:, :, :].rearrange("p a b -> p (a b)"))
                        nc.sync.dma_start(out=dbg["dbg_loc"][:, :], in_=o3[:, :])

                    # ---- C projection MAIN (no cross-core carry yet) ----
                    cp_ps = [ps4.tile([128, TSEG], f32, tag="acc",
                                      name="cp_ps") for _ in range(ND)]
                    for dd in range(ND):
                        for ss in range(NS):
                            nc.tensor.matmul(
                                cp_ps[dd][:, :],
                                CwT_sb[ss][:, dd * 128:(dd + 1) * 128],
                                hsT[ss][:, :],
                                start=(ss == 0), stop=False)

                    # ---- Dp*u + its residual add (independent of carry) ----
                    for dd in range(ND):
                        eng = nc.vector if dd % 2 == 0 else nc.gpsimd
                        t2 = scr.tile([128, TSEG], f32, tag="t2du", name="t2du")
                        eng.tensor_tensor(
                            out=t2[:, :], in0=hc[dd][:, :], in1=rstd[:, :],
                            op=OP.mult)
                        du = scr.tile([128, TSEG], f32, tag="du", name="du")
                        eng.tensor_scalar(
                            out=du[:, :], in0=t2[:, :],
                            scalar1=lncol("Dpw", ll, dd),
                            scalar2=lncol("Dpb", ll, dd),
                            op0=OP.mult, op1=OP.add)
                        eng.tensor_tensor(
                            out=h[dd][:, :], in0=h[dd][:, :], in1=du[:, :],
                            op=OP.add)

                    # ---- cross-core carry folded into the C projection ----
                    # GallT[p, ss*NCH+c] = g_out[c, ss*128+p] (transposed DMA)
                    GallT = one.tile([128, NS * NCH], f32, tag="GallT",
                                     name="GallT")
                    for ss in range(NS):
                        nc.sync.dma_start(
                            out=GallT[:, ss * NCH:(ss + 1) * NCH],
                            in_=bass.AP(tensor=g_out.tensor,
                                        offset=g_out.offset + ss * 128,
                                        ap=[[1, 128], [S, NCH]]))
                    rowsET = scr.tile([128, NS, NCH], f32, tag="rowsET",
                                      name="rowsET")
                    nc.vector.tensor_tensor(
                        out=rowsET[:, :, :],
                        in0=GallT[:, :].rearrange("p (a b) -> p a b", a=NS),
                        in1=segcT_sb[:, :].rearrange("p (a b) -> p a b", a=NS),
                        op=OP.mult)
                    # E[s] = sum_c rowsET[s, ss, c] — total incoming carry
                    ET = scr.tile([128, NS], f32, tag="ET", name="ET")
                    nc.vector.tensor_reduce(
                        out=ET[:, :], in_=rowsET[:, :, :],
                        axis=mybir.AxisListType.X, op=OP.add)
                    ctmp_ss = []
                    for ss in range(NS):
                        ctmp = scr.tile([128, TSEG], f32r, tag=f"ctmp{ss}",
                                        name=f"ctmp{ss}")
                        eng = nc.vector if ss % 2 == 0 else nc.gpsimd
                        eng.tensor_scalar_mul(
                            out=ctmp[:, :], in0=chc2T_sb[ss][:, :],
                            scalar1=ET[:, ss:ss + 1])
                        ctmp_ss.append(ctmp)
                    for dd in range(ND):
                        for ss in range(NS):
                            nc.tensor.matmul(
                                cp_ps[dd][:, :],
                                CwT_sb[ss][:, dd * 128:(dd + 1) * 128],
                                ctmp_ss[ss][:, :],
                                start=False, stop=(ss == NS - 1))
                    for dd in range(ND):
                        nc.vector.tensor_tensor(
                            out=h[dd][:, :], in0=h[dd][:, :], in1=cp_ps[dd][:, :],
                            op=OP.add)

                    if debug and ll == 0:
                        dump_fm("dbg_h1", h)

                    # ---- LN2 ----
                    xn2 = layer_norm("n2w", "n2b", ll, out_tag="xm")

                    # ---- SwiGLU ----
                    sw_ps = [ps4.tile([128, TSEG], f32, tag="acc", name="sw_ps")
                             for _ in range(ND)]
                    f0 = 0
                    for fi, pf in enumerate(FTS):
                        a_ps = ps.tile([128, TSEG], f32, tag="pa", name="a_ps")
                        b_ps = ps.tile([128, TSEG], f32, tag="pb", name="b_ps")
                        for dd in range(ND):
                            nc.tensor.matmul(
                                a_ps[:pf, :], w1sb[dd][:, f0:f0 + pf],
                                xn2[dd][:, :],
                                start=(dd == 0), stop=(dd == ND - 1))
                        for dd in range(ND):
                            nc.tensor.matmul(
                                b_ps[:pf, :], w2sb[dd][:, f0:f0 + pf],
                                xn2[dd][:, :],
                                start=(dd == 0), stop=(dd == ND - 1))
                        sa = scr.tile([128, TSEG], f32r, tag="sa", name="sa")
                        nc.scalar.activation(out=sa[:pf, :], in_=a_ps[:pf, :],
                                             func=AF.Silu)
                        g = scr.tile([128, TSEG], f32r, tag="g", name="g")
                        nc.vector.tensor_tensor(
                            out=g[:pf, :], in0=sa[:pf, :], in1=b_ps[:pf, :],
                            op=OP.mult)
                        w3t = scr.tile([128, D], f32r, tag="w3t", name="w3t",
                                       bufs=3)
                        nc.sync.dma_start(
                            out=w3t[:pf, :], in_=d_w3T[ll, f0:f0 + pf, :])
                        for dd in range(ND):
                            nc.tensor.matmul(
                                sw_ps[dd][:, :],
                                w3t[:pf, dd * 128:(dd + 1) * 128],
                                g[:pf, :],
                                start=(fi == 0), stop=(fi == NFT - 1))
                        f0 += pf
                    for dd in range(ND):
                        nc.vector.tensor_tensor(
                            out=h[dd][:, :], in0=h[dd][:, :], in1=sw_ps[dd][:, :],
                            op=OP.add)

            # ------- final LN per token chunk; gather launches per chunk ---
            NG = 4
            GCH = TSEG // NG  # gather chunk size in tokens
            xall_c = []
            xf_g = [dram.tile([D, GCH], bf16, tag=f"xf{g}", name=f"xf{g}")
                    for g in range(NG)]
            for tt in range(NCH):
                cs = slice(tt * CH, (tt + 1) * CH)
                mu = ps.tile([128, CH], f32, tag="sm", name="mu")
                ex2 = ps.tile([128, CH], f32, tag="gps", name="ex2")
                for dd in range(ND):
                    nc.tensor.matmul(mu[:, :], onesD_sb[:, :], h[dd][:, cs],
                                     start=(dd == 0), stop=(dd == ND - 1))
                for dd in range(ND):
                    h2 = scr.tile([128, CH], f32r, tag="h2f", name="h2f",
                                  bufs=2)
                    eng = nc.vector if dd % 2 == 0 else nc.gpsimd
                    eng.tensor_tensor(
                        out=h2[:, :], in0=h[dd][:, cs], in1=h[dd][:, cs],
                        op=OP.mult)
                    nc.tensor.matmul(ex2[:, :], onesD_sb[:, :], h2[:, :],
                                     start=(dd == 0), stop=(dd == ND - 1))
                mu_sbf = one.tile([128, CH], f32, tag="mu_sbf", name="mu_sbf")
                nc.scalar.activation(out=mu_sbf[:, :], in_=mu[:, :],
                                     func=AF.Copy)
                var = one.tile([128, CH], f32, tag="var_f", name="var_f")
                nc.vector.tensor_tensor(
                    out=var[:, :], in0=mu[:, :], in1=mu_sbf[:, :], op=OP.mult)
                nc.vector.tensor_tensor(
                    out=var[:, :], in0=ex2[:, :], in1=var[:, :], op=OP.subtract)
                sd = one.tile([128, CH], f32, tag="sd_f", name="sd_f")
                nc.scalar.activation(out=sd[:, :], in_=var[:, :], func=AF.Sqrt,
                                     bias=eps_sb[:, :], scale=1.0)
                rstd = one.tile([128, CH], f32, tag="rstd_f", name="rstd_f")
                nc.vector.reciprocal(out=rstd[:, :], in_=sd[:, :])
                TPG = NCH // NG  # LN chunks per gather chunk
                xf = xf_g[tt // TPG]
                c0 = (tt % TPG) * CH
                for dd in range(ND):
                    eng = nc.vector if dd % 2 == 0 else nc.gpsimd
                    hcf = scr.tile([128, CH], f32, tag="hcf", name="hcf",
                                   bufs=4)
                    eng.tensor_tensor(
                        out=hcf[:, :], in0=h[dd][:, cs], in1=mu_sbf[:, :],
                        op=OP.subtract)
                    t1 = scr.tile([128, CH], f32, tag="t1f", name="t1f",
                                  bufs=4)
                    eng.tensor_tensor(
                        out=t1[:, :], in0=hcf[:, :], in1=rstd[:, :], op=OP.mult)
                    xo = scr.tile([128, CH], bf16, tag=f"xof{tt}_{dd}",
                                  name=f"xof{tt}_{dd}")
                    eng.tensor_scalar(
                        out=xo[:, :], in0=t1[:, :],
                        scalar1=lncol("noww", None, dd),
                        scalar2=lncol("nob", None, dd),
                        op0=OP.mult, op1=OP.add)
                    nc.sync.dma_start(
                        out=xf[dd * 128:(dd + 1) * 128, c0:c0 + CH],
                        in_=xo[:, :])
                if tt % TPG == TPG - 1:
                    xall = dram.tile([NCORES * D, GCH], bf16,
                                     tag=f"xall{tt // TPG}",
                                     name=f"xall{tt // TPG}",
                                     addr_space="Shared")
                    nc.gpsimd.collective_compute(
                        "AllGather", mybir.AluOpType.bypass,
                        replica_groups=[list(range(NCORES))],
                        ins=[xf.opt()], outs=[xall.opt()],
                    )
                    xall_c.append(xall)

            act_ctx.__exit__(None, None, None)

            # ---------------- head (vocab-sharded, tt-pipelined) ----------
            NB = NCORES * ND  # 32 feature-row blocks of the gathered acts
            with tc.tile_pool(name="hdc", bufs=1) as hdc, \
                 tc.tile_pool(name="hd", bufs=2) as hd:
                # weight + bias loads overlap the first gather's flight
                hbb_sb = hdc.tile([128, VSH], f32, name="hbb_sb")
                nc.sync.dma_start(out=hbb_sb[:, :], in_=d_hbb[:, :])
                hw_all = []
                for vc in range(NVC):
                    row = []
                    for dd in range(ND):
                        t = hdc.tile([128, VC], bf16, tag=f"hwa{vc}_{dd}",
                                     name=f"hwa{vc}_{dd}")
                        nc.sync.dma_start(
                            out=t[:, :],
                            in_=d_hWTb[dd * 128:(dd + 1) * 128,
                                       vc * VC:(vc + 1) * VC])
                        row.append(t)
                    hw_all.append(row)
                for tt in range(NCH):
                    src = xall_c[tt // (NCH // NG)]
                    c0 = (tt % (NCH // NG)) * CH
                    xbl = []
                    for blk in range(NCORES):
                        t = hd.tile([128, ND * CH], bf16, tag=f"xb{blk}",
                                    name=f"xb{blk}")
                        nc.sync.dma_start(
                            out=t[:, :],
                            in_=bass.AP(
                                tensor=src.tensor,
                                offset=src.offset + blk * D * GCH + c0,
                                ap=[[GCH, 128], [128 * GCH, ND], [1, CH]]))
                        xbl.append(t)
                    for blk in range(NCORES):
                        for vc in range(NVC):
                            hp_ps = ps4.tile([128, TSEG], f32, tag="acc",
                                             name="hp_ps")
                            for dd in range(ND):
                                nc.tensor.matmul(
                                    hp_ps[:, :VC],
                                    xbl[blk][:, dd * CH:(dd + 1) * CH],
                                    hw_all[vc][dd][:, :],
                                    start=(dd == 0), stop=(dd == ND - 1))
                            ot = hd.tile([128, VC], bf16, tag="ot", name="ot",
                                         bufs=12)
                            nc.vector.tensor_tensor(
                                out=ot[:, :], in0=hp_ps[:, :VC],
                                in1=hbb_sb[:, vc * VC:(vc + 1) * VC],
                                op=OP.add)
                            t0 = blk * TSEG + tt * 128
                            nc.scalar.dma_start(
                                out=d_out[t0:t0 + 128,
                                          vc * VC:(vc + 1) * VC],
                                in_=ot[:, :])

    nc.compile()
    return nc


def _host_prep(inputs):
    """Build the 8 per-core input maps from full inputs."""
    x = np.asarray(inputs["x"]).astype(np.int32)
    emb = np.asarray(inputs["emb"], np.float32)
    pos = np.asarray(inputs["pos"], np.float32)
    lam = 1.0 / (1.0 + np.exp(-np.asarray(inputs["log_lambda"], np.float64)))
    Bw = np.asarray(inputs["Bw"], np.float32)
    Cw = np.asarray(inputs["Cw"], np.float32)
    w1 = np.asarray(inputs["w1"], np.float32)
    w2 = np.asarray(inputs["w2"], np.float32)
    w3 = np.asarray(inputs["w3"], np.float32)
    headW = np.asarray(inputs["headW"], np.float32)
    headb = np.asarray(inputs["headb"], np.float32)

    BwT = np.ascontiguousarray(Bw.transpose(0, 2, 1))
    CwT = np.ascontiguousarray(Cw.transpose(0, 2, 1))
    w1T = np.ascontiguousarray(w1.transpose(0, 2, 1))
    w2T = np.ascontiguousarray(w2.transpose(0, 2, 1))
    w3T = np.ascontiguousarray(w3.transpose(0, 2, 1))

    # packed LN params:
    # [n1w(L), n1b(L), n2w(L), n2b(L), Dp*n1w(L), now, nob, Dp*n1b(L)]
    Dp = np.asarray(inputs["Dp"], np.float32)
    n1w = np.asarray(inputs["n1w"], np.float32)
    n1b = np.asarray(inputs["n1b"], np.float32)
    lncols = np.zeros((D, NLC), np.float32)
    for i, arr in enumerate((n1w, n1b,
                             np.asarray(inputs["n2w"], np.float32),
                             np.asarray(inputs["n2b"], np.float32),
                             Dp * n1w)):
        lncols[:, i * L:(i + 1) * L] = arr.T
    lncols[:, 5 * L] = np.asarray(inputs["now"], np.float32)
    lncols[:, 5 * L + 1] = np.asarray(inputs["nob"], np.float32)
    lncols[:, 5 * L + 2:6 * L + 2] = (Dp * n1b).T

    i_ar = np.arange(CH, dtype=np.float64)[None, :, None]  # [1, CH, 1]
    lamB = lam[:, None, :]                                 # [L, 1, S]
    laminv = (lamB ** (-i_ar)).astype(np.float32)
    lamp = (lamB ** i_ar).astype(np.float32)
    chc = (lamB ** (i_ar + 1)).astype(np.float32)
    lamw = np.zeros((L, NCH, S), np.float32)
    for c in range(NCH):
        lamw[:, c, :] = (lam ** (TSEG - 1 - CH * c)).astype(np.float32)
    lamwB = np.ascontiguousarray(
        np.broadcast_to(lamw[:, None, :, :], (L, CH, NCH, S)), np.float32)
    t_ar = np.arange(TSEG, dtype=np.float64)[None, None, :]  # [1, 1, T]
    chc2T = (lam[:, :, None] ** (t_ar + 1)).astype(np.float32)  # [L, S, T]
    lamc = np.zeros((L, NCH, NCH, S), np.float32)
    for c in range(1, NCH):
        for cp in range(c):
            lamc[:, cp, c, :] = (lam ** (CH * (c - 1 - cp))).astype(np.float32)
    U = np.triu(np.ones((CH, CH), np.float32))
    ones4 = np.ones((NCH, 128), np.float32)
    ones4f = np.ones((NCH, TSEG), np.float32)
    ones128r = np.ones((128, 128), np.float32)
    onesD = np.full((128, 128), 1.0 / D, np.float32)

    in_maps = []
    for k in range(NCORES):
        b, r = divmod(k, NSEG)
        t0 = r * TSEG
        segcoef = np.zeros((L, NCH, S), np.float32)
        for sp in range(r):
            segcoef[:, sp, :] = (lam ** (TSEG * (r - 1 - sp))).astype(np.float32)
        # segcT[l, p, ss*NCH+c] = segcoef[l, c, ss*128+p]
        segcT = np.ascontiguousarray(
            segcoef.reshape(L, NCH, NS, 128).transpose(0, 3, 2, 1)
            .reshape(L, 128, NS * NCH))
        v0 = k * VSH
        in_maps.append({
            "x_seg": np.ascontiguousarray(x[b, t0:t0 + TSEG, None]),
            "emb": emb,
            "pos_seg": np.ascontiguousarray(pos[t0:t0 + TSEG]),
            "BwT": BwT, "CwT": CwT, "w1T": w1T, "w2T": w2T, "w3T": w3T,
            "headWTb": np.ascontiguousarray(
                headW[v0:v0 + VSH].T).astype(ml_dtypes.bfloat16),
            "headb_bc": np.ascontiguousarray(
                np.broadcast_to(headb[v0:v0 + VSH], (128, VSH))),
            "lncols": lncols,
            "laminv": laminv, "lamp": lamp, "chc": chc, "lamwB": lamwB,
            "chc2T": chc2T, "lamc": lamc, "segcoefT": segcT,
            "Utri": U, "ones4": ones4, "ones4f": ones4f,
            "ones128r": ones128r, "onesD": onesD,
        })
    return in_maps


def kernel(**inputs) -> np.ndarray:
    from concourse.bass_utils import run_bass_kernel_spmd

    if "nc" not in _NC_CACHE:
        _NC_CACHE["nc"] = _build_nc()
    nc = _NC_CACHE["nc"]
    in_maps = _host_prep(inputs)
    res = None
    last_err = None
    for _attempt in range(3):
        try:
            res = run_bass_kernel_spmd(nc, in_maps, core_ids=list(range(NCORES)))
            break
        except Exception as e:  # transient device hiccups: retry
            last_err = e
    if res is None:
        raise last_err
    parts = [np.asarray(res.results[k]["logits"]).astype(np.float32)
             for k in range(NCORES)]
    full = np.concatenate(parts, axis=1)
    return full.reshape(B, T, V)

